# revision 60
# baseline (speedup 1.0000x reference)
"""GF(2) linear block encoder c = (b @ G) mod 2 on 8 TRN2 NeuronCores.

Strategy:
  - Data-parallel: shard b rows (32768 -> 8 x 4096), replicate G.
  - Bits {0,1} are exact in fp8-e4m3 and products accumulate exactly in
    fp32 PSUM, so the GF(2) matmul is an fp8 DoubleRow matmul (K=256 per
    MM). HW floor: 216ns per 512-col DR matmul (1 col/cycle @2.4GHz),
    512 MMs/core = 110.6us of PE streaming; no faster PE mode exists
    (DoubleRow/DoubleColumn are the only 2x perf modes, packing output
    columns into wider dtypes exactly cancels DR's 2x because the PE is
    moving-operand-byte-bound).
  - Output is uint8 bits (ACT casts PSUM fp32 -> uint16, DVE ands with
    1 and casts to uint8 - DVE mod and fused and+convert are rejected
    by the compiler), half-rows ship as produced on a 3-queue rotation,
    host upcasts to int32.
  - Head: the framework preamble ends ~6.6us; each dma_start costs
    ~0.65us of issue time and first data lands ~8.2us. Critical pieces
    (b chunk 0 split in half on the otherwise-idle scalar queue, G
    512-col quarter slices striped kp-wise across sync/gpsimd) are
    pushed first. Zeroed 512-col warmup matmuls bridge the supply
    window and drive the DVFS ramp (PE runs at 1.2GHz until ~3us of
    sustained wide load); single warmups fill the two early data seams.
  - Tail: last two m-tiles extract per 512-col PSUM bank so the final
    chain is one quarter extract + one 64KiB DMA on emptied queues.
  - The whole run is near the DMA substrate's limit (~25-50
    descriptors/us/queue, ~120KB/us aggregate under 8-core contention;
    the scalar/ACT queue degrades further while ACT runs), so exec time
    = last-matmul (~127us) + extract chain + final flush + fixed
    drain/barrier epilogue (~3.7us). Deeper restructures (fused phases,
    2-bit packing, p-major block outputs) all measured SLOWER because
    their output scheduling left end-of-program queue backlogs.
    Measurements swing several us with chip thermal state (throttle
    activity_1 6% -> ~132.5us, 11% -> ~137us+).
"""

import sys

import numpy as np

if "/opt/trn_rl_repo" not in sys.path:
    sys.path.insert(0, "/opt/trn_rl_repo")

import ml_dtypes

B_ROWS = 32768
K_MSG = 1024
N_CODE = 2048
NCORES = 8
M = B_ROWS // NCORES  # 4096 rows per core
KS = K_MSG // 128     # 8 k-subtiles of 128
KP = KS // 2          # 4 DoubleRow k-pair steps (K=256 each)
MT = M // 128         # 32 m-tiles
MC = 16               # b chunks along m (2 m-tiles each)
MCW = M // MC         # 256 rows per chunk
BG = 4                # b chunks per group tile
NBG = MC // BG        # 4 groups

F8 = ml_dtypes.float8_e4m3

_NC_CACHE = None


def _build_bass():
    import concourse.bacc as bacc
    import concourse.mybir as mybir
    from concourse import tile

    nc = bacc.Bacc("TRN2", target_bir_lowering=False, debug=False)

    # bt[p, c, s, j] = b bit for row m = c*MCW + j, k = s*128 + p
    bt = nc.dram_tensor("bt", [128, MC, KS, MCW], mybir.dt.float8e4, kind="ExternalInput")
    g = nc.dram_tensor("g", [128, KS, N_CODE], mybir.dt.float8e4, kind="ExternalInput")
    c = nc.dram_tensor("c", [M, N_CODE], mybir.dt.uint8, kind="ExternalOutput")

    dr = mybir.MatmulPerfMode.DoubleRow
    NH = N_CODE // 2

    with tile.TileContext(nc) as tc:
        with (
            tc.tile_pool(name="persist", bufs=1) as persist,
            tc.tile_pool(name="psum", bufs=4, space="PSUM") as psum_pool,
            tc.tile_pool(name="mids", bufs=8) as mids,
            tc.tile_pool(name="c8s", bufs=8) as c8s,
        ):
            g_tiles = [
                persist.tile([128, 2, N_CODE], mybir.dt.float8e4, name=f"gt{kp}", tag=f"g{kp}")
                for kp in range(KP)
            ]
            b_groups = [
                persist.tile([128, BG, KS, MCW], mybir.dt.float8e4, name=f"bg{i}", tag=f"bg{i}")
                for i in range(NBG)
            ]

            def gq(kp, q, eng):
                # one 512-col quarter of one kp pair of G (128 KiB)
                eng.dma_start(
                    out=g_tiles[kp][:, :, q * 512 : (q + 1) * 512],
                    in_=g[:, 2 * kp : 2 * kp + 2, q * 512 : (q + 1) * 512],
                )

            def gh1(kp, eng):
                # n-half 1 of one kp pair (256 KiB), needed only in phase 1
                eng.dma_start(
                    out=g_tiles[kp][:, :, NH:],
                    in_=g[:, 2 * kp : 2 * kp + 2, NH:],
                )

            def bc(ch, eng):
                # one 256-row b chunk (256 KiB) feeding m-tiles 2ch, 2ch+1
                eng.dma_start(
                    out=b_groups[ch // BG][:, ch % BG : ch % BG + 1],
                    in_=bt[:, ch : ch + 1],
                )

            # --- input pushes, consumption-ordered. Critical path for the
            # first PSUM bank: b chunk 0 (split so kp0/kp1 land first) on
            # scalar, G q0 quarters striped kp-wise across sync/gpsimd.
            nc.scalar.dma_start(
                out=b_groups[0][:, 0:1, 0:4], in_=bt[:, 0:1, 0:4]
            )
            gq(0, 0, nc.sync)
            gq(1, 0, nc.gpsimd)
            nc.scalar.dma_start(
                out=b_groups[0][:, 0:1, 4:8], in_=bt[:, 0:1, 4:8]
            )
            gq(2, 0, nc.sync)
            gq(3, 0, nc.gpsimd)
            bc(1, nc.scalar)
            gq(0, 1, nc.sync)
            gq(1, 1, nc.gpsimd)
            gq(2, 1, nc.sync)
            gq(3, 1, nc.gpsimd)
            for ch in (3, 5, 7, 9):
                bc(ch, nc.sync)
            for ch in (2, 4, 6, 8, 10):
                bc(ch, nc.gpsimd)
            gh1(0, nc.sync)
            gh1(2, nc.sync)
            gh1(1, nc.gpsimd)
            gh1(3, nc.gpsimd)
            for ch in (11, 13, 15):
                bc(ch, nc.sync)
            for ch in (12, 14):
                bc(ch, nc.gpsimd)

            # --- PE warmups: full-width 512-col matmuls on a zeroed dummy
            # tile into a dead PSUM bank. A tiny tile memsets first so the
            # earliest warmups start ~6.8us; the 512-col ones drive the
            # DVFS ramp while the first input DMAs fly.
            zw0 = persist.tile([128, 2, 128], mybir.dt.float8e4, name="zw0")
            zw = persist.tile([128, 2, 512], mybir.dt.float8e4, name="zwarm")
            nc.vector.memset(zw0, 0)
            nc.vector.memset(zw, 0)
            ps_warm = psum_pool.tile([128, NH], mybir.dt.float32, name="ps")

            def warm(cols=512):
                src = zw0 if cols <= 128 else zw
                nc.tensor.matmul(
                    ps_warm[:, 0:cols],
                    src[:, :, 0:128],
                    src[:, :, 0:cols],
                    start=True,
                    stop=True,
                    perf_mode=dr,
                )

            for _ in range(4):
                warm(64)
            for _ in range(4):
                warm(512)

            # output viewed per m-tile: m = mt*128 + p
            c_view = c.rearrange("(mt p) n -> mt p n", p=128)

            out_eng = [nc.gpsimd, nc.sync, nc.scalar]

            def bsta(mt, kp):
                mc, j = mt // 2, mt % 2
                return b_groups[mc // BG][
                    :, mc % BG, 2 * kp : 2 * kp + 2, j * 128 : (j + 1) * 128
                ]

            def extract(mid, c8, ps, s0, s1):
                nc.scalar.activation(
                    mid[:, s0:s1], ps, mybir.ActivationFunctionType.Copy
                )
                nc.vector.tensor_scalar(
                    out=mid[:, s0:s1], in0=mid[:, s0:s1], scalar1=1,
                    scalar2=None, op0=mybir.AluOpType.bitwise_and,
                )
                nc.vector.tensor_scalar(
                    out=c8[:, s0:s1], in0=mid[:, s0:s1], scalar1=0,
                    scalar2=None, op0=mybir.AluOpType.bypass,
                )

            for ph in range(2):
                n0 = ph * NH
                for mt in range(MT):
                    head_mode = ph == 0 and mt < 2
                    quarter_mode = ph == 1 and mt >= MT - 2
                    if head_mode:
                        # per-quarter PSUM banks, kp-ordered to match DMA
                        # arrival; warmup fillers bridge the data seams
                        mid = mids.tile([128, NH], mybir.dt.uint16)
                        c8 = c8s.tile([128, NH], mybir.dt.uint8)
                        for q in range(2):
                            psq = psum_pool.tile([128, 512], mybir.dt.float32, name="ps")
                            for kp in range(KP):
                                nc.tensor.matmul(
                                    psq,
                                    bsta(mt, kp),
                                    g_tiles[kp][:, :, q * 512 : (q + 1) * 512],
                                    start=(kp == 0),
                                    stop=(kp == KP - 1),
                                    perf_mode=dr,
                                )
                                if mt == 0 and q == 0 and kp == 1:
                                    warm(512)  # fill the kp1->kp2 data seam
                            extract(mid, c8, psq, q * 512, (q + 1) * 512)
                        if mt == 0:
                            warm(512)  # fill the q0->q1 data seam
                        nc.gpsimd.dma_start(out=c_view[mt][:, 0:NH], in_=c8)
                    elif not quarter_mode:
                        ps = psum_pool.tile([128, NH], mybir.dt.float32, name="ps")
                        for kp in range(KP):
                            for nt in range(2):
                                nc.tensor.matmul(
                                    ps[:, nt * 512 : (nt + 1) * 512],
                                    bsta(mt, kp),
                                    g_tiles[kp][:, :, n0 + nt * 512 : n0 + (nt + 1) * 512],
                                    start=(kp == 0),
                                    stop=(kp == KP - 1),
                                    perf_mode=dr,
                                )
                        mid = mids.tile([128, NH], mybir.dt.uint16)
                        c8 = c8s.tile([128, NH], mybir.dt.uint8)
                        extract(mid, c8, ps, 0, NH)
                        out_eng[(ph * MT + mt) % 3].dma_start(
                            out=c_view[mt][:, n0 : n0 + NH], in_=c8
                        )
                    else:
                        # final two half-tiles: per-bank PSUM quarters so the
                        # tail is one 512-col extract chain + one 64 KiB DMA
                        # on queues that have gone idle
                        mid = mids.tile([128, NH], mybir.dt.uint16)
                        c8 = c8s.tile([128, NH], mybir.dt.uint8)
                        qrings = {(MT - 2, 0): nc.gpsimd, (MT - 2, 1): nc.sync,
                                  (MT - 1, 0): nc.sync, (MT - 1, 1): nc.scalar}
                        for nt in range(2):
                            psq = psum_pool.tile([128, 512], mybir.dt.float32, name="ps")
                            for kp in range(KP):
                                nc.tensor.matmul(
                                    psq,
                                    bsta(mt, kp),
                                    g_tiles[kp][:, :, n0 + nt * 512 : n0 + (nt + 1) * 512],
                                    start=(kp == 0),
                                    stop=(kp == KP - 1),
                                    perf_mode=dr,
                                )
                            s0, s1 = nt * 512, (nt + 1) * 512
                            extract(mid, c8, psq, s0, s1)
                            qrings[(mt, nt)].dma_start(
                                out=c_view[mt][:, n0 + s0 : n0 + s1],
                                in_=c8[:, s0:s1],
                            )

    nc.finalize()
    return nc


def _get_nc():
    global _NC_CACHE
    if _NC_CACHE is None:
        _NC_CACHE = _build_bass()
    return _NC_CACHE


def _pack_inputs(b, G):
    b8 = np.asarray(b).astype(np.uint8)
    G8 = np.asarray(G).astype(np.uint8)
    # g[p, s, n], k = s*128 + p
    g_f8 = G8.reshape(KS, 128, N_CODE).transpose(1, 0, 2).astype(F8, order="C")
    bts = []
    for core in range(NCORES):
        sh = b8[core * M : (core + 1) * M]  # [M, K]
        # bt[p, c, s, j]: m = c*MCW + j, k = s*128 + p
        btc = sh.reshape(MC, MCW, KS, 128).transpose(3, 0, 2, 1)
        bts.append(btc.astype(F8, order="C"))
    return bts, g_f8


def kernel(b, G, trace=False, **run_kwargs):
    from concourse.bass_utils import run_bass_kernel_spmd

    nc = _get_nc()
    bts, g_f8 = _pack_inputs(b, G)
    in_maps = [{"bt": bts[i], "g": g_f8} for i in range(NCORES)]
    res = run_bass_kernel_spmd(
        nc, in_maps, core_ids=list(range(NCORES)), trace=trace, **run_kwargs
    )
    out = np.concatenate([res.results[i]["c"] for i in range(NCORES)], axis=0)
    out = out.astype(np.int32)
    if trace:
        kernel.last_results = res
    return out


kernel.last_results = None
